# revision 3
# baseline (speedup 1.0000x reference)
"""Self-contained Trainium2 Bass kernel for nn_Decoder_79809082294812.

kernel(**inputs) takes the FULL unsharded inputs (embeddings [1024,1000,128],
remaining_capacity [1024], Wqg [257,128], Wkg/Wvg/Wog/Wqo/Wko [128,128],
current_node [1024], mask [1024,1000]) and returns (probs, logits), each
[1024, 1000] float32 — matching the reference decoder.

Sharding: pure data-parallel over the batch dim across 8 NeuronCores
(128 batch elements per core); weights replicated.

Algebraic restructuring vs the reference (exact, fp32):
  compat[b,h,n] = emb[b,n,:] @ U_b[:,h],  U_b[:,h] = (Wkg[:,hD:(h+1)D]/sqrt(D)) @ q_b[h]
  A[b,:,h]      = sum_n attn[b,h,n] emb[b,n,:]
  heads[b,hd]   = sum_e Wvg[e,hd] A[b,e,h(hd)]
  comp[b,n]     = emb[b,n,:] @ w_b,        w_b = (Wko @ qo_b)/sqrt(E)
so the reference's [N,E]@[E,E] projections (k, v, ko) are never materialized;
each batch element's embeddings stream through the PE a few times against
tiny per-b vectors.  This makes the kernel memory-bound as intended.
"""
import contextlib
import ctypes
import math
import os
import sys
import types

sys.path.insert(0, '/opt/trn_rl_repo')

from contextlib import ExitStack
import numpy as np

import concourse.bass as bass
import concourse.tile as tile
from concourse import bacc, mybir
from concourse.bass_utils import run_bass_kernel_spmd

F32 = mybir.dt.float32
AF = mybir.ActivationFunctionType
AX = mybir.AxisListType
ALU = mybir.AluOpType

B = 1024
N = 1000
E = 128
H = 8
D = 16
CTX = 2 * E + 1
N_CORES = 8
BC = B // N_CORES   # batch elements per core
NCH = 8             # n-chunks per batch element
CH = 125            # rows per chunk (8*125 = 1000)
G = 8               # group size (batch elements per group)

WNAME_SHAPES = {
    "wqg_mean": [E, E], "wqg_cur": [E, E], "wqg_cap": [1, E],
    "wkgT_blk": [E, H * E], "wvg": [E, E], "wog": [E, E], "wqo": [E, E],
    "wkoT": [E, E], "m8rep": [E, G * H], "ident": [128, 128],
}

_NC_CACHE = {}
LAST_RESULT = None   # BassKernelResults of the most recent run (for profiling)


# --------------------------------------------------------------------------
# Optional NTFF profiling hook (enabled only when BASS_TRACE is set).  The
# agent image's antenv lacks axon_hooks; this registers the ctypes
# equivalent so run_bass_kernel_spmd(trace=True) can return exec_time_ns.
# --------------------------------------------------------------------------
def _install_profile_shim():
    so_path = '/opt/axon/libaxon_pjrt.so'
    try:
        import antenv
    except ImportError:
        return
    if 'antenv.axon_hooks' not in sys.modules:
        mod = types.ModuleType('antenv.axon_hooks')
        mod._hook = None

        def set_axon_ntff_profile_hook(h):
            mod._hook = h

        def get_axon_ntff_profile_hook():
            return mod._hook

        mod.set_axon_ntff_profile_hook = set_axon_ntff_profile_hook
        mod.get_axon_ntff_profile_hook = get_axon_ntff_profile_hook
        sys.modules['antenv.axon_hooks'] = mod
        antenv.axon_hooks = mod
    mod = sys.modules['antenv.axon_hooks']
    if mod.get_axon_ntff_profile_hook() is not None:
        return
    try:
        lib = ctypes.CDLL(so_path)
    except OSError:
        return
    if not hasattr(lib, "axon_start_nrt_profile"):
        return
    lib.axon_start_nrt_profile.argtypes = [ctypes.POINTER(ctypes.c_int64),
                                           ctypes.c_size_t]
    lib.axon_start_nrt_profile.restype = ctypes.c_int64
    lib.axon_stop_nrt_profile.argtypes = [ctypes.c_char_p]
    lib.axon_stop_nrt_profile.restype = ctypes.c_int64

    @contextlib.contextmanager
    def _hook(output_dir, device_ids):
        import jax
        jax.devices()
        if device_ids:
            ids = (ctypes.c_int64 * len(device_ids))(*device_ids)
            rc = lib.axon_start_nrt_profile(ids, len(device_ids))
        else:
            rc = lib.axon_start_nrt_profile(None, 0)
        if rc != 0:
            raise RuntimeError(f"axon_start_nrt_profile rc={rc}")
        try:
            yield
        finally:
            n = lib.axon_stop_nrt_profile(str(output_dir).encode())
            if n < 0:
                raise RuntimeError(f"axon_stop_nrt_profile rc={n}")

    mod.set_axon_ntff_profile_hook(_hook)
    import concourse.bass_utils as bu
    bu.upload_artifacts = lambda tmpdir: f"local:{tmpdir}"


def _host_prep_weights(Wqg, Wkg, Wvg, Wog, Wqo, Wko):
    w = {}
    w["wqg_mean"] = np.ascontiguousarray(Wqg[:E] / float(N)).astype(np.float32)
    w["wqg_cur"] = np.ascontiguousarray(Wqg[E:2 * E]).astype(np.float32)
    w["wqg_cap"] = np.ascontiguousarray(Wqg[2 * E:2 * E + 1]).astype(np.float32)
    wkgT = (Wkg / math.sqrt(D)).T.astype(np.float32)        # [hd, e]
    blk = np.zeros((E, H, E), np.float32)                   # [hd, h, e]
    for h in range(H):
        blk[h * D:(h + 1) * D, h, :] = wkgT[h * D:(h + 1) * D, :]
    w["wkgT_blk"] = np.ascontiguousarray(blk.reshape(E, H * E))
    w["wvg"] = np.ascontiguousarray(Wvg).astype(np.float32)
    w["wog"] = np.ascontiguousarray(Wog).astype(np.float32)
    w["wqo"] = np.ascontiguousarray(Wqo).astype(np.float32)
    w["wkoT"] = np.ascontiguousarray((Wko / math.sqrt(E)).T).astype(np.float32)
    m8 = np.zeros((E, H), np.float32)
    for hd in range(E):
        m8[hd, hd // D] = 1.0
    w["m8rep"] = np.ascontiguousarray(np.tile(m8, (1, G))).astype(np.float32)
    w["ident"] = np.eye(128, dtype=np.float32)
    return w


def _build_nc(Bc=BC, n_devices=N_CORES):
    assert Bc % G == 0
    NG = Bc // G
    nc = bacc.Bacc("TRN2", target_bir_lowering=False, debug=False,
                   num_devices=n_devices)

    emb = nc.dram_tensor("emb", [Bc, N, E], F32, kind="ExternalInput").ap()
    curT = nc.dram_tensor("curT", [E, Bc], F32, kind="ExternalInput").ap()
    cap = nc.dram_tensor("cap", [1, Bc], F32, kind="ExternalInput").ap()
    wap = {k: nc.dram_tensor(k, s, F32, kind="ExternalInput").ap()
           for k, s in WNAME_SHAPES.items()}
    probs_out = nc.dram_tensor("probs", [Bc, N], F32, kind="ExternalOutput").ap()
    logits_out = nc.dram_tensor("logits", [Bc, N], F32, kind="ExternalOutput").ap()

    with tile.TileContext(nc) as tc, ExitStack() as ctx:
        # ---- constants ----
        cpool = ctx.enter_context(tc.tile_pool(name="consts", bufs=1))
        w_sb = {}
        for k, s in WNAME_SHAPES.items():
            t = cpool.tile(s, F32, tag=k)
            nc.sync.dma_start(t[:], wap[k][:])
            w_sb[k] = t
        curT_sb = cpool.tile([E, Bc], F32, tag="curT")
        nc.sync.dma_start(curT_sb[:], curT[:])
        cap_sb = cpool.tile([1, Bc], F32, tag="cap")
        nc.sync.dma_start(cap_sb[:], cap[:])
        ident = w_sb["ident"]

        # ---- pools ----
        nat_pool = ctx.enter_context(tc.tile_pool(name="nat", bufs=2 * G))
        embT_pool = ctx.enter_context(tc.tile_pool(name="embT", bufs=2 * G))
        attn_pool = ctx.enter_context(tc.tile_pool(name="attn", bufs=2))
        attnT_pool = ctx.enter_context(tc.tile_pool(name="attnT", bufs=2))
        sm_pool = ctx.enter_context(tc.tile_pool(name="smalls", bufs=3))
        stage_pool = ctx.enter_context(tc.tile_pool(name="stage", bufs=1))

        # PSUM budget (8 banks): ptr 1 + pcT 2 + pcN 2 + pA 1 + ps 2
        ptr_pool = ctx.enter_context(tc.tile_pool(name="ptr", bufs=1, space="PSUM"))
        pcT_pool = ctx.enter_context(tc.tile_pool(name="pcT", bufs=2, space="PSUM"))
        pcN_pool = ctx.enter_context(tc.tile_pool(name="pcN", bufs=1, space="PSUM"))
        pA_pool = ctx.enter_context(tc.tile_pool(name="pA", bufs=1, space="PSUM"))
        ps_pool = ctx.enter_context(tc.tile_pool(name="ps", bufs=2, space="PSUM"))

        t_stage = stage_pool.tile([128, N], F32, tag="t")
        p_stage = stage_pool.tile([128, N], F32, tag="p")

        for g in range(NG):
            nat = []
            embT = []
            gs0 = sm_pool.tile([E, G], F32, tag="gs0")
            gs1 = sm_pool.tile([E, G], F32, tag="gs1")
            for i in range(G):
                b = g * G + i
                nb = nat_pool.tile([CH, NCH, E], F32, tag="nat")
                nc.sync.dma_start(
                    nb[:], emb[b].rearrange("(c p) e -> p c e", p=CH))
                nat.append(nb)
                eb = embT_pool.tile([E, N], F32, tag="embT")
                for t2 in range(2):
                    pt = ptr_pool.tile([128, 4 * CH], F32, tag="ptr")
                    for jj in range(4):
                        j = t2 * 4 + jj
                        nc.tensor.transpose(
                            pt[:, jj * CH:(jj + 1) * CH],
                            nb[:, j, :],
                            ident[:CH, :CH])
                    gs = gs0 if t2 == 0 else gs1
                    nc.scalar.activation(
                        eb[:, t2 * 4 * CH:(t2 + 1) * 4 * CH], pt[:],
                        AF.Copy, accum_out=gs[:, i:i + 1])
                embT.append(eb)

            # qT [128q, G] = Wqg.T @ context  (mean folded into wqg_mean)
            pqT = ps_pool.tile([E, G], F32, tag="ps")
            nc.tensor.matmul(pqT[:], w_sb["wqg_mean"][:], gs0[:],
                             start=True, stop=False)
            nc.tensor.matmul(pqT[:], w_sb["wqg_mean"][:], gs1[:],
                             start=False, stop=False)
            nc.tensor.matmul(pqT[:], w_sb["wqg_cur"][:],
                             curT_sb[:, g * G:(g + 1) * G],
                             start=False, stop=False)
            nc.tensor.matmul(pqT[:], w_sb["wqg_cap"][:],
                             cap_sb[:, g * G:(g + 1) * G],
                             start=False, stop=True)
            qT_sb = sm_pool.tile([E, G], F32, tag="qT")
            nc.scalar.copy(qT_sb[:], pqT[:])

            # U [128e, H, G]
            pU = ps_pool.tile([E, H * G], F32, tag="ps")
            wkgT_blk = w_sb["wkgT_blk"][:].rearrange("p (h e) -> p h e", h=H)
            for h in range(H):
                nc.tensor.matmul(
                    pU[:, h * G:(h + 1) * G],
                    wkgT_blk[:, h, :],
                    qT_sb[:],
                    start=True, stop=True)
            U_sb = sm_pool.tile([E, H, G], F32, tag="U")
            nc.scalar.copy(U_sb[:], pU[:])

            # compatT [125, (chunk, b, h)]
            pcT = pcT_pool.tile([CH, NCH * G * H], F32, tag="pcT")
            for i in range(G):
                for j in range(NCH):
                    nc.tensor.matmul(
                        pcT[:, (j * G + i) * H:(j * G + i) * H + H],
                        embT[i][:, j * CH:(j + 1) * CH],
                        U_sb[:, :, i],
                        start=True, stop=True)
            cT_sb = sm_pool.tile([CH, NCH, G * H], F32, tag="cT")
            nc.scalar.copy(cT_sb[:], pcT[:])

            # compat rows [G*H, chunk, 125] (bank-padded chunk slots)
            pcN = pcN_pool.tile([G * H, NCH, 128], F32, tag="pcN")
            for j in range(NCH):
                nc.tensor.transpose(
                    pcN[:, j, :CH],
                    cT_sb[:, j, :],
                    ident[:CH, :CH])

            # softmax over n, batched across G*H rows
            negmax = sm_pool.tile([G * H, 1], F32, tag="negmax")
            nc.vector.tensor_reduce(negmax[:], pcN[:, :, :CH], axis=AX.XY,
                                    op=ALU.max, negate=True)
            attn = attn_pool.tile([G * H, N], F32, tag="attn")
            attn_v = attn[:].rearrange("p (c n) -> p c n", n=CH)
            sums = sm_pool.tile([G * H, 1], F32, tag="sums")
            nc.scalar.activation(attn_v, pcN[:, :, :CH], AF.Exp,
                                 bias=negmax[:], accum_out=sums[:])
            recip = sm_pool.tile([G * H, 1], F32, tag="recip")
            nc.vector.reciprocal(recip[:], sums[:])
            nc.vector.tensor_scalar_mul(attn[:], attn[:], recip[:])

            # attnT [125, chunk, G*H]
            attnT = attnT_pool.tile([CH, NCH, G * H], F32, tag="attnT")
            for j in range(NCH):
                pat = ps_pool.tile([CH, G * H], F32, tag="ps")
                nc.tensor.transpose(
                    pat[:], attn[:, j * CH:(j + 1) * CH], ident[:G * H, :G * H])
                nc.scalar.copy(attnT[:, j, :], pat[:])

            # A [128e, G*H]
            pA = pA_pool.tile([E, G * H], F32, tag="pA")
            for i in range(G):
                for j in range(NCH):
                    nc.tensor.matmul(
                        pA[:, i * H:(i + 1) * H],
                        nat[i][:, j, :],
                        attnT[:, j, i * H:(i + 1) * H],
                        start=(j == 0), stop=(j == NCH - 1))
            A_sb = sm_pool.tile([E, G * H], F32, tag="A")
            nc.scalar.copy(A_sb[:], pA[:])

            # heads [128hd, G]
            pheads = ps_pool.tile([E, G * H], F32, tag="ps")
            nc.tensor.matmul(pheads[:], w_sb["wvg"][:], A_sb[:],
                             start=True, stop=True)
            tmp = sm_pool.tile([E, G * H], F32, tag="tmp")
            nc.vector.tensor_mul(tmp[:], pheads[:], w_sb["m8rep"][:])
            heads = sm_pool.tile([E, G], F32, tag="heads")
            nc.vector.reduce_sum(
                heads[:], tmp[:].rearrange("p (g h) -> p g h", h=H), axis=AX.X)

            # glimpseT, qoT, w
            pgl = ps_pool.tile([E, G], F32, tag="ps")
            nc.tensor.matmul(pgl[:], w_sb["wog"][:], heads[:], start=True, stop=True)
            gl_sb = sm_pool.tile([E, G], F32, tag="gl")
            nc.scalar.copy(gl_sb[:], pgl[:])

            pqo = ps_pool.tile([E, G], F32, tag="ps")
            nc.tensor.matmul(pqo[:], w_sb["wqo"][:], gl_sb[:], start=True, stop=True)
            qo_sb = sm_pool.tile([E, G], F32, tag="qo")
            nc.scalar.copy(qo_sb[:], pqo[:])

            pw = ps_pool.tile([E, G], F32, tag="ps")
            nc.tensor.matmul(pw[:], w_sb["wkoT"][:], qo_sb[:], start=True, stop=True)
            w_vec = sm_pool.tile([E, G], F32, tag="w")
            nc.scalar.copy(w_vec[:], pw[:])

            # compT [125, chunk, G], re-transpose per chunk, tanh into t_stage
            pcc = pcT_pool.tile([CH, NCH * G], F32, tag="pcT")
            for i in range(G):
                for j in range(NCH):
                    nc.tensor.matmul(
                        pcc[:, j * G + i:j * G + i + 1],
                        embT[i][:, j * CH:(j + 1) * CH],
                        w_vec[:, i:i + 1],
                        start=True, stop=True)
            cc_sb = sm_pool.tile([CH, NCH, G], F32, tag="cc")
            nc.scalar.copy(cc_sb[:], pcc[:])
            tg = sm_pool.tile([G, N], F32, tag="tg")
            for j in range(NCH):
                pc2 = ps_pool.tile([G, CH], F32, tag="ps")
                nc.tensor.transpose(pc2[:], cc_sb[:, j, :], ident[:CH, :CH])
                nc.scalar.activation(
                    tg[:, j * CH:(j + 1) * CH], pc2[:], AF.Tanh)
            # engines need 32-aligned partition bases; DMA does not
            nc.sync.dma_start(t_stage[g * G:(g + 1) * G, :], tg[:])

        # epilogue: logits = 10*tanh; probs = softmax(logits)
        nc.vector.tensor_scalar_mul(t_stage[:Bc, :], t_stage[:Bc, :], 10.0)
        nc.sync.dma_start(logits_out[:], t_stage[:Bc, :])
        negmax2 = stage_pool.tile([128, 1], F32, tag="negmax2")
        nc.vector.tensor_reduce(negmax2[:Bc], t_stage[:Bc, :], axis=AX.X,
                                op=ALU.max, negate=True)
        sums2 = stage_pool.tile([128, 1], F32, tag="sums2")
        nc.scalar.activation(p_stage[:Bc, :], t_stage[:Bc, :], AF.Exp,
                             bias=negmax2[:Bc], accum_out=sums2[:Bc])
        recip2 = stage_pool.tile([128, 1], F32, tag="recip2")
        nc.vector.reciprocal(recip2[:Bc], sums2[:Bc])
        nc.vector.tensor_scalar_mul(p_stage[:Bc, :], p_stage[:Bc, :], recip2[:Bc])
        nc.sync.dma_start(probs_out[:], p_stage[:Bc, :])

    nc.compile()
    return nc


def _get_nc():
    key = (BC, N_CORES)
    if key not in _NC_CACHE:
        _NC_CACHE[key] = _build_nc(*key)
    return _NC_CACHE[key]


def kernel(embeddings, remaining_capacity, Wqg, Wkg, Wvg, Wog, Wqo, Wko,
           current_node, mask):
    global LAST_RESULT
    embeddings = np.asarray(embeddings, dtype=np.float32)
    remaining_capacity = np.asarray(remaining_capacity, dtype=np.float32)
    Wqg = np.asarray(Wqg, dtype=np.float32)
    Wkg = np.asarray(Wkg, dtype=np.float32)
    Wvg = np.asarray(Wvg, dtype=np.float32)
    Wog = np.asarray(Wog, dtype=np.float32)
    Wqo = np.asarray(Wqo, dtype=np.float32)
    Wko = np.asarray(Wko, dtype=np.float32)
    current_node = np.asarray(current_node)
    mask = np.asarray(mask)
    assert embeddings.shape == (B, N, E)

    trace = bool(os.environ.get("BASS_TRACE"))
    if trace:
        _install_profile_shim()

    w = _host_prep_weights(Wqg, Wkg, Wvg, Wog, Wqo, Wko)
    cur = embeddings[np.arange(B), current_node.astype(np.int64)]  # [B, E]
    curT = np.ascontiguousarray(cur.T)                             # [E, B]
    cap = remaining_capacity[None, :]                              # [1, B]

    nc = _get_nc()
    in_maps = []
    for c in range(N_CORES):
        sl = slice(c * BC, (c + 1) * BC)
        m = {
            "emb": np.ascontiguousarray(embeddings[sl]),
            "curT": np.ascontiguousarray(curT[:, sl]),
            "cap": np.ascontiguousarray(cap[:, sl]),
        }
        m.update(w)
        in_maps.append(m)

    kw = {}
    if trace:
        kw = dict(trace=True, trace_cores=[0])
    res = run_bass_kernel_spmd(nc, in_maps, list(range(N_CORES)), **kw)
    LAST_RESULT = res

    probs = np.concatenate([res.results[c]["probs"] for c in range(N_CORES)], 0)
    logits = np.concatenate([res.results[c]["logits"] for c in range(N_CORES)], 0)

    if mask.any():
        # General-correctness slow path (the spec always sends an all-False
        # mask): the mask affects the glimpse attention too, so recompute
        # everything for the masked rows on the host.
        probs, logits = _numpy_full(embeddings, remaining_capacity, Wqg, Wkg,
                                    Wvg, Wog, Wqo, Wko, cur, mask)

    return probs.astype(np.float32), logits.astype(np.float32)


def _numpy_full(emb, capv, Wqg, Wkg, Wvg, Wog, Wqo, Wko, cur, mask):
    graph = emb.mean(axis=1)
    context = np.concatenate([graph, cur, capv[:, None]], axis=-1)
    q = (context @ Wqg).reshape(B, H, D)
    k = (emb @ Wkg).reshape(B, N, H, D)
    v = (emb @ Wvg).reshape(B, N, H, D)
    compat = np.einsum('bhd,bnhd->bhn', q, k) / math.sqrt(D)
    compat = np.where(mask[:, None, :], -np.inf, compat)
    m = compat.max(axis=-1, keepdims=True)
    a = np.exp(compat - m)
    attn = a / a.sum(axis=-1, keepdims=True)
    heads = np.einsum('bhn,bnhd->bhd', attn, v).reshape(B, E)
    glimpse = heads @ Wog
    qo = glimpse @ Wqo
    ko = emb @ Wko
    comp = np.einsum('be,bne->bn', qo, ko) / math.sqrt(E)
    logits = 10.0 * np.tanh(comp)
    logits = np.where(mask, -np.inf, logits)
    m2 = logits.max(axis=-1, keepdims=True)
    a2 = np.exp(logits - m2)
    probs = a2 / a2.sum(axis=-1, keepdims=True)
    return probs.astype(np.float32), logits.astype(np.float32)


# revision 11
# speedup vs baseline: 3.5075x; 3.5075x over previous
"""Self-contained Trainium2 Bass kernel for nn_Decoder_79809082294812.

kernel(**inputs) takes the FULL unsharded inputs (embeddings [1024,1000,128],
remaining_capacity [1024], Wqg [257,128], Wkg/Wvg/Wog/Wqo/Wko [128,128],
current_node [1024], mask [1024,1000]) and returns (probs, logits), each
[1024, 1000] float32 — matching the reference decoder.

Sharding: pure data-parallel over the batch dim across 8 NeuronCores
(128 batch elements per core); weights replicated.

Algebraic restructuring vs the reference:
  compat[b,h,n] = emb[b,n,:] @ U_b[:,h],  U_b[:,h] = (Wkg[:,hD:(h+1)D]/sqrt(D)) @ q_b[h]
  A[b,:,h]      = sum_n attn[b,h,n] emb[b,n,:]
  heads[b,hd]   = sum_e Wvg[e,hd] A[b,e,h(hd)]
  w_b           = Wbig @ heads_b   (Wog/Wqo/Wko chain folded on host)
  comp[b,n]     = emb[b,n,:] @ w_b
so the reference's [N,E]@[E,E] projections (k, v, ko) are never materialized;
each batch element's embeddings stream through the PE a few times against
tiny per-b vectors.  Embeddings travel in bf16 (fp32 accumulation in PSUM);
the small per-batch chain (q/U/heads/w) stays fp32.
"""
import contextlib
import ctypes
import math
import os
import sys
import types

sys.path.insert(0, '/opt/trn_rl_repo')

from contextlib import ExitStack
import numpy as np
import ml_dtypes

import concourse.bass as bass
import concourse.tile as tile
from concourse import bacc, mybir
from concourse.bass_utils import run_bass_kernel_spmd

F32 = mybir.dt.float32
BF16 = mybir.dt.bfloat16
AF = mybir.ActivationFunctionType
AX = mybir.AxisListType
ALU = mybir.AluOpType
BF16_NP = ml_dtypes.bfloat16

B = 1024
N = 1000
E = 128
H = 8
D = 16
CTX = 2 * E + 1
N_CORES = 8
BC = B // N_CORES   # batch elements per core
NCH = 8             # n-chunks per batch element (n = 8p + c)
CH = 125            # rows per chunk
G = 8               # group size; 2 subgroups of 4 share a [128,1000] psum
SPLIT = 512         # psum-bank-aligned split of the n axis

WNAME_SHAPES = {
    "wqg_mean": ([E, E], F32), "wqg_cur": ([E, E], F32), "wqg_cap": ([1, E], F32),
    "wkgT": ([E, E], F32), "wvg": ([E, E], F32), "wbig": ([E, E], F32),
    "m8rep": ([E, G * H], F32), "maskHG": ([E, G * H], F32),
    "identb": ([128, 128], BF16),
}

_NC_CACHE = {}
LAST_RESULT = None   # BassKernelResults of the most recent run (for profiling)


# --------------------------------------------------------------------------
# Optional NTFF profiling hook (enabled only when BASS_TRACE is set).  The
# agent image's antenv lacks axon_hooks; this registers the ctypes
# equivalent so run_bass_kernel_spmd(trace=True) can return exec_time_ns.
# --------------------------------------------------------------------------
def _install_profile_shim():
    so_path = '/opt/axon/libaxon_pjrt.so'
    try:
        import antenv
    except ImportError:
        return
    if 'antenv.axon_hooks' not in sys.modules:
        mod = types.ModuleType('antenv.axon_hooks')
        mod._hook = None

        def set_axon_ntff_profile_hook(h):
            mod._hook = h

        def get_axon_ntff_profile_hook():
            return mod._hook

        mod.set_axon_ntff_profile_hook = set_axon_ntff_profile_hook
        mod.get_axon_ntff_profile_hook = get_axon_ntff_profile_hook
        sys.modules['antenv.axon_hooks'] = mod
        antenv.axon_hooks = mod
    mod = sys.modules['antenv.axon_hooks']
    if mod.get_axon_ntff_profile_hook() is not None:
        return
    try:
        lib = ctypes.CDLL(so_path)
    except OSError:
        return
    if not hasattr(lib, "axon_start_nrt_profile"):
        return
    lib.axon_start_nrt_profile.argtypes = [ctypes.POINTER(ctypes.c_int64),
                                           ctypes.c_size_t]
    lib.axon_start_nrt_profile.restype = ctypes.c_int64
    lib.axon_stop_nrt_profile.argtypes = [ctypes.c_char_p]
    lib.axon_stop_nrt_profile.restype = ctypes.c_int64

    @contextlib.contextmanager
    def _hook(output_dir, device_ids):
        import jax
        jax.devices()
        if device_ids:
            ids = (ctypes.c_int64 * len(device_ids))(*device_ids)
            rc = lib.axon_start_nrt_profile(ids, len(device_ids))
        else:
            rc = lib.axon_start_nrt_profile(None, 0)
        if rc != 0:
            raise RuntimeError(f"axon_start_nrt_profile rc={rc}")
        try:
            yield
        finally:
            n = lib.axon_stop_nrt_profile(str(output_dir).encode())
            if n < 0:
                raise RuntimeError(f"axon_stop_nrt_profile rc={n}")

    mod.set_axon_ntff_profile_hook(_hook)
    import concourse.bass_utils as bu
    bu.upload_artifacts = lambda tmpdir: f"local:{tmpdir}"


def _host_prep_weights(Wqg, Wkg, Wvg, Wog, Wqo, Wko):
    w = {}
    w["wqg_mean"] = np.ascontiguousarray(Wqg[:E] / float(N)).astype(np.float32)
    w["wqg_cur"] = np.ascontiguousarray(Wqg[E:2 * E]).astype(np.float32)
    w["wqg_cap"] = np.ascontiguousarray(Wqg[2 * E:2 * E + 1]).astype(np.float32)
    w["wkgT"] = np.ascontiguousarray((Wkg / math.sqrt(D)).T).astype(np.float32)
    w["wvg"] = np.ascontiguousarray(Wvg).astype(np.float32)
    # w_b = Wbig.T @ heads_b with Wbig[c,e] = (Wog @ Wqo @ Wko.T)[c,e]/sqrt(E)
    w["wbig"] = np.ascontiguousarray(
        (Wog @ Wqo @ Wko.T) / math.sqrt(E)).astype(np.float32)
    # head-extraction mask over (i, h) lanes: [hd, i*H + h]
    m8 = np.zeros((E, H), np.float32)
    for hd in range(E):
        m8[hd, hd // D] = 1.0
    w["m8rep"] = np.ascontiguousarray(np.tile(m8, (1, G))).astype(np.float32)
    # U-expansion mask over (h, i) lanes: [hd, h*G + i] = (hd//D == h)
    mhg = np.zeros((E, H, G), np.float32)
    for hd in range(E):
        mhg[hd, hd // D, :] = 1.0
    w["maskHG"] = np.ascontiguousarray(mhg.reshape(E, H * G))
    w["identb"] = np.eye(128, dtype=BF16_NP)
    return w


def _build_nc(Bc=BC, n_devices=N_CORES):
    assert Bc % G == 0
    NG = Bc // G
    nc = bacc.Bacc("TRN2", target_bir_lowering=False, debug=False,
                   num_devices=n_devices)

    emb = nc.dram_tensor("emb", [Bc, N, E], BF16, kind="ExternalInput").ap()
    embT = nc.dram_tensor("embT", [Bc, E, N], BF16, kind="ExternalInput").ap()
    curT = nc.dram_tensor("curT", [E, Bc], F32, kind="ExternalInput").ap()
    cap = nc.dram_tensor("cap", [1, Bc], F32, kind="ExternalInput").ap()
    wap = {k: nc.dram_tensor(k, s, dt, kind="ExternalInput").ap()
           for k, (s, dt) in WNAME_SHAPES.items()}
    probs_out = nc.dram_tensor("probs", [Bc, N], F32, kind="ExternalOutput").ap()
    logits_out = nc.dram_tensor("logits", [Bc, N], F32, kind="ExternalOutput").ap()

    with tile.TileContext(nc) as tc, ExitStack() as ctx:
        # ---- constants ----
        cpool = ctx.enter_context(tc.tile_pool(name="consts", bufs=1))
        w_sb = {}
        for k, (s, dt) in WNAME_SHAPES.items():
            t = cpool.tile(s, dt, tag=k)
            nc.sync.dma_start(t[:], wap[k][:])
            w_sb[k] = t
        curT_sb = cpool.tile([E, Bc], F32, tag="curT")
        nc.sync.dma_start(curT_sb[:], curT[:])
        cap_sb = cpool.tile([1, Bc], F32, tag="cap")
        nc.sync.dma_start(cap_sb[:], cap[:])

        # ---- pools ----
        nat_pool = ctx.enter_context(tc.tile_pool(name="nat", bufs=3 * G))
        embT_pool = ctx.enter_context(tc.tile_pool(name="embT", bufs=3 * G))
        attn_pool = ctx.enter_context(tc.tile_pool(name="attn", bufs=4))
        attnT_pool = ctx.enter_context(tc.tile_pool(name="attnT", bufs=4))
        tg_pool = ctx.enter_context(tc.tile_pool(name="tg", bufs=2))
        sm_pool = ctx.enter_context(tc.tile_pool(name="smalls", bufs=3))
        stage_pool = ctx.enter_context(tc.tile_pool(name="stage", bufs=1))

        # PSUM (8 banks): pcm 2x2 + pA 1x2 + ps 1x2
        pcm_pool = ctx.enter_context(tc.tile_pool(name="pcm", bufs=2, space="PSUM"))
        pA_pool = ctx.enter_context(tc.tile_pool(name="pA", bufs=2, space="PSUM"))
        ps_pool = ctx.enter_context(tc.tile_pool(name="ps", bufs=2, space="PSUM"))

        t_stage = stage_pool.tile([128, N], F32, tag="t")
        p_stage = stage_pool.tile([128, N], F32, tag="p")
        scratch = stage_pool.tile([128, N - SPLIT], BF16, tag="scr")

        for g in range(NG):
            nat = []
            embTs = []
            gs0 = sm_pool.tile([E, G], F32, tag="gs0")
            gs1 = sm_pool.tile([E, G], F32, tag="gs1")
            for i in range(G):
                b = g * G + i
                nb = nat_pool.tile([CH, NCH, E], BF16, tag="nat")
                nc.sync.dma_start(
                    nb[:], emb[b].rearrange("(p c) e -> p c e", c=NCH))
                nat.append(nb)
                eb = embT_pool.tile([E, N], BF16, tag="embT")
                nc.sync.dma_start(eb[:], embT[b])
                embTs.append(eb)
                # mean (as sums; 1/N folded into wqg_mean): DVE half + ACT half
                nc.vector.reduce_sum(gs0[:, i:i + 1], eb[:, :SPLIT], axis=AX.X)
                nc.scalar.activation(scratch[:], eb[:, SPLIT:], AF.Copy,
                                     accum_out=gs1[:, i:i + 1])

            # ---- qT [128q, G] = Wqg.T @ context ----
            gsum = sm_pool.tile([E, G], F32, tag="gsum")
            nc.vector.tensor_add(gsum[:], gs0[:], gs1[:])
            pqT = ps_pool.tile([E, G], F32, tag="ps")
            nc.tensor.matmul(pqT[:], w_sb["wqg_mean"][:], gsum[:],
                             start=True, stop=False)
            nc.tensor.matmul(pqT[:], w_sb["wqg_cur"][:],
                             curT_sb[:, g * G:(g + 1) * G],
                             start=False, stop=False)
            nc.tensor.matmul(pqT[:], w_sb["wqg_cap"][:],
                             cap_sb[:, g * G:(g + 1) * G],
                             start=False, stop=True)
            qT_sb = sm_pool.tile([E, G], F32, tag="qT")
            nc.scalar.copy(qT_sb[:], pqT[:])

            # ---- U [128e, (h,i)] via one matmul over broadcast-masked qT ----
            qT_exp = sm_pool.tile([E, H, G], F32, tag="qTe")
            qt_ap = qT_sb[:]
            qt_b = bass.AP(qt_ap.tensor, qt_ap.offset,
                           [list(qt_ap.ap[0]), [0, H], [1, G]])
            nc.vector.tensor_mul(
                qT_exp[:], qt_b,
                w_sb["maskHG"][:].rearrange("p (h i) -> p h i", h=H))
            pU = ps_pool.tile([E, H * G], F32, tag="ps")
            nc.tensor.matmul(pU[:], w_sb["wkgT"][:],
                             qT_exp[:].rearrange("p h i -> p (h i)"),
                             start=True, stop=True)
            # [E, G, 32] zero-padded so compat writes full 32-row blocks
            U32 = sm_pool.tile([E, G, 32], BF16, tag="U32")
            nc.gpsimd.memset(U32[:], 0.0)
            nc.scalar.copy(U32[:, :, 0:H],
                           pU[:].rearrange("p (h i) -> p i h", h=H))

            # ---- compat: per subgroup of 4, [128,1000] psum (rows 32*p') ----
            pcms = []
            for k in range(2):
                # free-size 1024 keeps partition strides bank-aligned
                pcm = pcm_pool.tile([128, 1024], F32, tag="pcm")
                pcms.append(pcm)
                for pp in range(4):
                    i = 4 * k + pp
                    for s0, s1 in ((0, SPLIT), (SPLIT, N)):
                        nc.tensor.matmul(
                            pcm[32 * pp:32 * pp + 32, s0:s1],
                            U32[:, i, :],
                            embTs[i][:, s0:s1],
                            start=True, stop=True,
                            tile_position=(0, 32 * pp))

            # ---- softmax + attnT per subgroup ----
            attnTs = []
            for k in range(2):
                pcm = pcms[k]
                negmax = sm_pool.tile([128, 1], F32, tag="negmax")
                nc.vector.tensor_reduce(negmax[:], pcm[:, :N], axis=AX.X,
                                        op=ALU.max, negate=True)
                attn = attn_pool.tile([128, N], BF16, tag="attn")
                sums = sm_pool.tile([128, 1], F32, tag="sums")
                nc.scalar.activation(attn[:], pcm[:, :N], AF.Exp,
                                     bias=negmax[:], accum_out=sums[:])
                recip = sm_pool.tile([128, 1], F32, tag="recip")
                nc.vector.reciprocal(recip[:], sums[:])
                nc.vector.tensor_scalar_mul(attn[:], attn[:], recip[:])
                # attnT [125, c, row]: chunk c holds nodes n = 8p + c
                attnT = attnT_pool.tile([CH, NCH, 128], BF16, tag="attnT")
                attn_v = attn[:].rearrange("r (p c) -> r c p", c=NCH)
                for c in range(NCH):
                    pat = ps_pool.tile([CH, 128], BF16, tag="ps")
                    nc.tensor.transpose(pat[:], attn_v[:, c, :],
                                        w_sb["identb"][:])
                    nc.scalar.copy(attnT[:, c, :], pat[:])
                attnTs.append(attnT)

            # ---- A [128e, (i,h)] ----
            pA = pA_pool.tile([E, G * H], F32, tag="pA")
            for i in range(G):
                k, pp = divmod(i, 4)
                for c in range(NCH):
                    nc.tensor.matmul(
                        pA[:, i * H:(i + 1) * H],
                        nat[i][:, c, :],
                        attnTs[k][:, c, 32 * pp:32 * pp + H],
                        start=(c == 0), stop=(c == NCH - 1))
            A_sb = sm_pool.tile([E, G * H], F32, tag="A")
            nc.scalar.copy(A_sb[:], pA[:])

            # ---- heads [128hd, G], then w = Wbig.T @ heads ----
            pheads = ps_pool.tile([E, G * H], F32, tag="ps")
            nc.tensor.matmul(pheads[:], w_sb["wvg"][:], A_sb[:],
                             start=True, stop=True)
            tmp = sm_pool.tile([E, G * H], F32, tag="tmp")
            nc.vector.tensor_mul(tmp[:], pheads[:], w_sb["m8rep"][:])
            heads = sm_pool.tile([E, G], F32, tag="heads")
            nc.vector.reduce_sum(
                heads[:], tmp[:].rearrange("p (g h) -> p g h", h=H), axis=AX.X)
            pw = ps_pool.tile([E, G], F32, tag="ps")
            nc.tensor.matmul(pw[:], w_sb["wbig"][:], heads[:],
                             start=True, stop=True)
            w32 = sm_pool.tile([E, G, 32], BF16, tag="w32")
            nc.gpsimd.memset(w32[:], 0.0)
            nc.scalar.copy(w32[:, :, 0:1], pw[:].rearrange("p (g o) -> p g o", o=1))

            # ---- comp + tanh, per subgroup ----
            for k in range(2):
                pcm2 = pcm_pool.tile([128, 1024], F32, tag="pcm")
                for pp in range(4):
                    i = 4 * k + pp
                    for s0, s1 in ((0, SPLIT), (SPLIT, N)):
                        nc.tensor.matmul(
                            pcm2[32 * pp:32 * pp + 32, s0:s1],
                            w32[:, i, :],
                            embTs[i][:, s0:s1],
                            start=True, stop=True,
                            tile_position=(0, 32 * pp))
                tgt = tg_pool.tile([128, N], F32, tag="tg")
                nc.scalar.activation(tgt[:], pcm2[:, :N], AF.Tanh)
                # move the 4 valid rows into t_stage (DMA: no 32-alignment)
                for pp in range(4):
                    nc.sync.dma_start(
                        t_stage[g * G + 4 * k + pp:g * G + 4 * k + pp + 1, :],
                        tgt[32 * pp:32 * pp + 1, :])

        # ---- epilogue: logits = 10*tanh; probs = softmax(logits) ----
        nc.vector.tensor_scalar_mul(t_stage[:Bc, :], t_stage[:Bc, :], 10.0)
        nc.sync.dma_start(logits_out[:], t_stage[:Bc, :])
        negmax2 = stage_pool.tile([128, 1], F32, tag="negmax2")
        nc.vector.tensor_reduce(negmax2[:Bc], t_stage[:Bc, :], axis=AX.X,
                                op=ALU.max, negate=True)
        sums2 = stage_pool.tile([128, 1], F32, tag="sums2")
        nc.scalar.activation(p_stage[:Bc, :], t_stage[:Bc, :], AF.Exp,
                             bias=negmax2[:Bc], accum_out=sums2[:Bc])
        recip2 = stage_pool.tile([128, 1], F32, tag="recip2")
        nc.vector.reciprocal(recip2[:Bc], sums2[:Bc])
        nc.vector.tensor_scalar_mul(p_stage[:Bc, :], p_stage[:Bc, :], recip2[:Bc])
        nc.sync.dma_start(probs_out[:], p_stage[:Bc, :])

    nc.compile()
    return nc


def _get_nc():
    key = (BC, N_CORES)
    if key not in _NC_CACHE:
        _NC_CACHE[key] = _build_nc(*key)
    return _NC_CACHE[key]


def kernel(embeddings, remaining_capacity, Wqg, Wkg, Wvg, Wog, Wqo, Wko,
           current_node, mask):
    global LAST_RESULT
    embeddings = np.asarray(embeddings, dtype=np.float32)
    remaining_capacity = np.asarray(remaining_capacity, dtype=np.float32)
    Wqg = np.asarray(Wqg, dtype=np.float32)
    Wkg = np.asarray(Wkg, dtype=np.float32)
    Wvg = np.asarray(Wvg, dtype=np.float32)
    Wog = np.asarray(Wog, dtype=np.float32)
    Wqo = np.asarray(Wqo, dtype=np.float32)
    Wko = np.asarray(Wko, dtype=np.float32)
    current_node = np.asarray(current_node)
    mask = np.asarray(mask)
    assert embeddings.shape == (B, N, E)

    trace = bool(os.environ.get("BASS_TRACE"))
    if trace:
        _install_profile_shim()

    w = _host_prep_weights(Wqg, Wkg, Wvg, Wog, Wqo, Wko)
    cur = embeddings[np.arange(B), current_node.astype(np.int64)]  # [B, E]
    curT = np.ascontiguousarray(cur.T)                             # [E, B]
    cap = remaining_capacity[None, :]                              # [1, B]
    emb_bf = embeddings.astype(BF16_NP)                            # [B, N, E]
    embT_bf = np.ascontiguousarray(emb_bf.transpose(0, 2, 1))      # [B, E, N]

    nc = _get_nc()
    in_maps = []
    for c in range(N_CORES):
        sl = slice(c * BC, (c + 1) * BC)
        m = {
            "emb": np.ascontiguousarray(emb_bf[sl]),
            "embT": embT_bf[sl],
            "curT": np.ascontiguousarray(curT[:, sl]),
            "cap": np.ascontiguousarray(cap[:, sl]),
        }
        m.update(w)
        in_maps.append(m)

    kw = {}
    if trace:
        kw = dict(trace=True, trace_cores=[0])
    res = run_bass_kernel_spmd(nc, in_maps, list(range(N_CORES)), **kw)
    LAST_RESULT = res

    probs = np.concatenate([res.results[c]["probs"] for c in range(N_CORES)], 0)
    logits = np.concatenate([res.results[c]["logits"] for c in range(N_CORES)], 0)

    if mask.any():
        # General-correctness slow path (the spec always sends an all-False
        # mask): the mask affects the glimpse attention too, so recompute
        # everything for the masked rows on the host.
        probs, logits = _numpy_full(embeddings, remaining_capacity, Wqg, Wkg,
                                    Wvg, Wog, Wqo, Wko, cur, mask)

    return probs.astype(np.float32), logits.astype(np.float32)


def _numpy_full(emb, capv, Wqg, Wkg, Wvg, Wog, Wqo, Wko, cur, mask):
    graph = emb.mean(axis=1)
    context = np.concatenate([graph, cur, capv[:, None]], axis=-1)
    q = (context @ Wqg).reshape(B, H, D)
    k = (emb @ Wkg).reshape(B, N, H, D)
    v = (emb @ Wvg).reshape(B, N, H, D)
    compat = np.einsum('bhd,bnhd->bhn', q, k) / math.sqrt(D)
    compat = np.where(mask[:, None, :], -np.inf, compat)
    m = compat.max(axis=-1, keepdims=True)
    a = np.exp(compat - m)
    attn = a / a.sum(axis=-1, keepdims=True)
    heads = np.einsum('bhn,bnhd->bhd', attn, v).reshape(B, E)
    glimpse = heads @ Wog
    qo = glimpse @ Wqo
    ko = emb @ Wko
    comp = np.einsum('be,bne->bn', qo, ko) / math.sqrt(E)
    logits = 10.0 * np.tanh(comp)
    logits = np.where(mask, -np.inf, logits)
    m2 = logits.max(axis=-1, keepdims=True)
    a2 = np.exp(logits - m2)
    probs = a2 / a2.sum(axis=-1, keepdims=True)
    return probs.astype(np.float32), logits.astype(np.float32)
